# revision 7
# baseline (speedup 1.0000x reference)
"""Trainium2 kernel for CondensedLinearFineGrainedSparseOp:
    out[b,s,o] = sum_k x[b,s,k] * weight[o,k] + bias[o]
with x [8, 2048, 4096] f32, weight [4096, 4096] f32 (90% zeros, stored
dense), bias [4096] f32 -> out [8, 2048, 4096] f32.

Strategy: data-parallel shard over tokens (B*S = 16384 -> 2048 per core)
across 8 NeuronCores; weight/bias replicated. The unstructured 10%
sparsity is not exploitable on the 128x128 PE array, so each core runs a
dense [2048 x 4096 x 4096] GEMM in bf16 with fp32 PSUM accumulation.
PE roofline: 16t*32k*4096o cols @ 0.4167ns = 874us/core.

Key layout decision: W is pre-tiled on host into one contiguous
partition-major blob per o-phase ([P, KT, olen], 32KB/partition lines).
DMA descriptor-processing overhead, not HBM bandwidth, limits how fast a
single queue delivers data; 1KB-line strided W reads from a [k, o] matrix
deliver ~10x slower than contiguous reads, which starved the PE for the
first ~30us of earlier schedules. With contiguous blobs the cold W chunk
(k0-1, 256KB) lands ~1us after queue start.

Schedule: each o-phase (512, 512, 1024, 1024, 1024 wide) processes t-tile
pairs; within a pair the k-loop interleaves (t, t+1) x (PSUM banks), so a
W k-tile feeds 868ns+ of matmul per touch and per-x-tile first-touch
stalls expose once per pair instead of once per tile. PSUM: 4 banks per
pair double-buffered. W blob for phase p+1 streams during phase p (pool
WAR + in-order rings time it); x tiles re-stream per phase on the SWDGE
queue; outputs alternate the two HWDGE rings behind the current phase's
W chunks but ahead of later phases'.
"""

import numpy as np
import ml_dtypes

import concourse.mybir as mybir
import concourse.tile as tile
from concourse import bacc
from concourse.bass import ts
from concourse.bass_utils import run_bass_kernel_spmd

P = 128
NCORES = 8
B, S, DIN, DOUT = 8, 2048, 4096, 4096
T = B * S // NCORES          # tokens per core
KT = DIN // P                # 32 contraction tiles
NT = T // P                  # 16 token tiles per core
BANK = 512                   # PSUM bank width (f32)
OBLK = 1024                  # W pool slot width

# (o0, olen) per phase; all phases interleave pairs of t-tiles
PHASE_PLAN = [
    (0, 512),
    (512, 512),
    (1024, 1024),
    (2048, 1024),
    (3072, 1024),
]

BF16 = mybir.dt.bfloat16
F32 = mybir.dt.float32

_NC = None
LAST_RESULT = None


def _build_nc():
    nc = bacc.Bacc("TRN2", target_bir_lowering=False, debug=False)
    # x pre-tiled on host: xt[t, p, ks, i] = x[t*128+i, ks*128+p]
    xt = nc.dram_tensor("xt", [NT, P, KT, P], BF16, kind="ExternalInput")
    # W pre-tiled per phase: w{p}[p_, k, o] = weight[o0+o, k*128+p_]
    wph = [
        nc.dram_tensor(f"w{p}", [P, KT, olen], BF16, kind="ExternalInput")
        for p, (o0, olen) in enumerate(PHASE_PLAN)
    ]
    bias = nc.dram_tensor("bias_rep", [P, DOUT], F32, kind="ExternalInput")
    out = nc.dram_tensor("out", [T, DOUT], F32, kind="ExternalOutput")

    with tile.TileContext(nc) as tc:
        with (
            tc.tile_pool(name="wpool", bufs=2) as wpool,
            tc.tile_pool(name="xpool", bufs=5) as xpool,
            tc.tile_pool(name="bpool", bufs=1) as bpool,
            tc.tile_pool(name="opool", bufs=3) as opool,
            tc.tile_pool(name="psum", bufs=8, space="PSUM") as psum_pool,
        ):
            # Tiny warmup DMA on each queue: absorbs cold DGE/queue init.
            for i, eng in enumerate((nc.sync, nc.scalar, nc.gpsimd)):
                wu = bpool.tile([P, 8], F32, tag=f"wu{i}", name=f"wu{i}")
                eng.dma_start(wu[:], bias.ap()[:, ts(i, 8)])

            bias_sb = bpool.tile([P, DOUT], F32)

            xt_sb = {}

            def x_prefetch(t):
                xtile = xpool.tile([P, KT, P], BF16, tag="x", name=f"x{t}")
                nc.gpsimd.dma_start(xtile[:], xt.ap()[t])
                xt_sb[t] = xtile

            # Cold first pair (t0, t1): quarter-chunks interleaved so
            # matmul (k=0, t=0) needs only 256KB of x.
            for t in (0, 1):
                xt_sb[t] = xpool.tile([P, KT, P], BF16, tag="x", name=f"x{t}")
            for c in range(4):
                for t in (0, 1):
                    nc.gpsimd.dma_start(
                        xt_sb[t][:, ts(c, 8), :], xt.ap()[t, :, ts(c, 8), :]
                    )
            x_prefetch(2)
            x_prefetch(3)

            wblk = [None] * len(PHASE_PLAN)

            def w_emit(p):
                o0, olen = PHASE_PLAN[p]
                blk = wpool.tile(
                    [P, KT, olen], BF16, tag="w", name=f"w{p}",
                    padded_shape=[P, KT, OBLK],
                )
                wblk[p] = blk
                # chunk in k so delivery tracks consumption; finer head
                # chunks on phase 0 put the first W tile in SBUF ~1us in
                chunks = (
                    [(0, 2), (2, 4), (4, 8), (8, 16), (16, 24), (24, 32)]
                    if p == 0
                    else [(0, 8), (8, 16), (16, 24), (24, 32)]
                )
                for i, (k0, k1) in enumerate(chunks):
                    eng = nc.sync if i % 2 == 0 else nc.scalar
                    eng.dma_start(
                        blk[:, k0:k1, :], wph[p].ap()[:, k0:k1, :]
                    )

            pairs = [
                (p, tg) for p in range(len(PHASE_PLAN))
                for tg in range(0, NT, 2)
            ]
            for gi, (p, tg) in enumerate(pairs):
                o0, olen = PHASE_PLAN[p]
                nb = olen // BANK
                tpair = (tg, tg + 1)
                if tg == 0:
                    # W blobs: phase 0+1 up front, then p+1 at each phase
                    # start (its pool-slot WAR makes it stream during p).
                    if p == 0:
                        w_emit(0)
                        w_emit(1)
                    elif p + 1 < len(PHASE_PLAN):
                        w_emit(p + 1)
                    nc.gpsimd.dma_start(
                        bias_sb[:, o0:o0 + olen], bias.ap()[:, o0:o0 + olen]
                    )

                # prefetch next pair's x tiles (re-streamed every phase)
                if gi + 1 < len(pairs):
                    for t in (pairs[gi + 1][1], pairs[gi + 1][1] + 1):
                        if t not in xt_sb:
                            x_prefetch(t)

                accs = {
                    t: [
                        psum_pool.tile([P, BANK], F32, tag="acc", name="acc")
                        for _ in range(nb)
                    ]
                    for t in tpair
                }
                for k in range(KT):
                    for t in tpair:
                        stat = xt_sb[t][:, k, :]
                        for b in range(nb):
                            nc.tensor.matmul(
                                accs[t][b][:],
                                stat,                            # stationary
                                wblk[p][:, k, ts(b, BANK)],      # moving
                                start=(k == 0),
                                stop=(k == KT - 1),
                            )
                for t in tpair:
                    osb = opool.tile(
                        [P, olen], F32, tag="o", name="o",
                        padded_shape=[P, OBLK],
                    )
                    for b in range(nb):
                        nc.vector.tensor_add(
                            osb[:, ts(b, BANK)],
                            accs[t][b][:],
                            bias_sb[:, o0 + b * BANK:o0 + (b + 1) * BANK],
                        )
                    # outs alternate rings; they sit ahead of later phases'
                    # W triggers in ring order so W WAR blocking never
                    # delays an output write
                    oeng = nc.sync if t % 2 == 0 else nc.scalar
                    oeng.dma_start(
                        out.ap()[ts(t, P), o0:o0 + olen], osb[:]
                    )
                for t in tpair:
                    del xt_sb[t]

    nc.compile()
    return nc


def kernel(x, weight, bias):
    global _NC, LAST_RESULT
    if _NC is None:
        _NC = _build_nc()

    X = np.ascontiguousarray(x.reshape(B * S, DIN))
    wt = weight.T.astype(ml_dtypes.bfloat16)          # [k, o] bf16
    # per-phase contiguous W blobs: [p_, k, o] = wt[k*128+p_, o0+o]
    wblobs = {}
    for p, (o0, olen) in enumerate(PHASE_PLAN):
        blk = wt[:, o0:o0 + olen].reshape(KT, P, olen).transpose(1, 0, 2)
        wblobs[f"w{p}"] = np.ascontiguousarray(blk)
    bias_rep = np.ascontiguousarray(
        np.broadcast_to(bias.astype(np.float32), (P, DOUT))
    )
    in_maps = []
    for c in range(NCORES):
        xc = X[c * T : (c + 1) * T].astype(ml_dtypes.bfloat16)
        # [t-tile, p(=k%128), ks, i(=token%128)]
        xt_c = np.ascontiguousarray(
            xc.reshape(NT, P, KT, P).transpose(0, 3, 2, 1)
        )
        m = {"xt": xt_c, "bias_rep": bias_rep}
        m.update(wblobs)
        in_maps.append(m)

    last_err = None
    for _attempt in range(2):
        try:
            res = run_bass_kernel_spmd(_NC, in_maps, list(range(NCORES)))
            break
        except Exception as e:  # transient NRT device errors: retry once
            last_err = e
    else:
        raise last_err
    LAST_RESULT = res

    out = np.concatenate([res.results[c]["out"] for c in range(NCORES)], axis=0)
    return out.reshape(B, S, DOUT).astype(np.float32, copy=False)


# revision 12
# speedup vs baseline: 1.0004x; 1.0004x over previous
"""Trainium2 kernel for CondensedLinearFineGrainedSparseOp:
    out[b,s,o] = sum_k x[b,s,k] * weight[o,k] + bias[o]
with x [8, 2048, 4096] f32, weight [4096, 4096] f32 (90% zeros, stored
dense), bias [4096] f32 -> out [8, 2048, 4096] f32.

Strategy: data-parallel shard over tokens (B*S = 16384 -> 2048 per core)
across 8 NeuronCores; weight/bias replicated. The unstructured 10%
sparsity is not exploitable on the 128x128 PE array, so each core runs a
dense [2048 x 4096 x 4096] GEMM in bf16 with fp32 PSUM accumulation.
PE roofline: 16t*32k*4096o cols @ 0.4167ns = 874us/core.

Key layout decision: W is pre-tiled on host into one contiguous
partition-major blob per o-phase ([P, KT, olen], 32KB/partition lines).
DMA descriptor-processing overhead, not HBM bandwidth, limits how fast a
single queue delivers data; 1KB-line strided W reads from a [k, o] matrix
deliver ~10x slower than contiguous reads, which starved the PE for the
first ~30us of earlier schedules. With contiguous blobs the cold W chunk
(k0-1, 256KB) lands ~1us after queue start.

Schedule: each o-phase (512, 512, 1024, 1024, 1024 wide) processes t-tile
pairs; within a pair the k-loop interleaves (t, t+1) x (PSUM banks), so a
W k-tile feeds 868ns+ of matmul per touch and per-x-tile first-touch
stalls expose once per pair instead of once per tile. PSUM: 4 banks per
pair double-buffered. W blob for phase p+1 streams during phase p (pool
WAR + in-order rings time it); x tiles re-stream per phase on the SWDGE
queue; outputs alternate the two HWDGE rings behind the current phase's
W chunks but ahead of later phases'.
"""

import numpy as np
import ml_dtypes

import concourse.mybir as mybir
import concourse.tile as tile
from concourse import bacc
from concourse.bass import ts
from concourse.bass_utils import run_bass_kernel_spmd

P = 128
NCORES = 8
B, S, DIN, DOUT = 8, 2048, 4096, 4096
T = B * S // NCORES          # tokens per core
KT = DIN // P                # 32 contraction tiles
NT = T // P                  # 16 token tiles per core
BANK = 512                   # PSUM bank width (f32)
OBLK = 1024                  # W pool slot width

# (o0, olen) per phase; all phases interleave pairs of t-tiles
PHASE_PLAN = [
    (0, 512),
    (512, 512),
    (1024, 1024),
    (2048, 1024),
    (3072, 1024),
]

BF16 = mybir.dt.bfloat16
F32 = mybir.dt.float32

_NC = None
LAST_RESULT = None


def _build_nc():
    nc = bacc.Bacc("TRN2", target_bir_lowering=False, debug=False)
    # x pre-tiled on host: xt[t, p, ks, i] = x[t*128+i, ks*128+p]
    xt = nc.dram_tensor("xt", [NT, P, KT, P], BF16, kind="ExternalInput")
    # W pre-tiled per phase into a contiguous partition-major blob:
    # w{p}[p_, k*olen + o] = weight[o0+o, k*128+p_]  (2D, 32KB lines)
    wph = [
        nc.dram_tensor(f"w{p}", [P, KT * olen], BF16, kind="ExternalInput")
        for p, (o0, olen) in enumerate(PHASE_PLAN)
    ]
    bias = nc.dram_tensor("bias_rep", [P, DOUT], F32, kind="ExternalInput")
    out = nc.dram_tensor("out", [T, DOUT], F32, kind="ExternalOutput")

    with tile.TileContext(nc) as tc:
        with (
            tc.tile_pool(name="wpool", bufs=8) as wpool,
            tc.tile_pool(name="xpool", bufs=5) as xpool,
            tc.tile_pool(name="bpool", bufs=1) as bpool,
            tc.tile_pool(name="opool", bufs=3) as opool,
            tc.tile_pool(name="psum", bufs=8, space="PSUM") as psum_pool,
        ):
            # Tiny warmup DMA on each queue: absorbs cold DGE/queue init.
            for i, eng in enumerate((nc.sync, nc.scalar, nc.gpsimd)):
                wu = bpool.tile([P, 8], F32, tag=f"wu{i}", name=f"wu{i}")
                eng.dma_start(wu[:], bias.ap()[:, ts(i, 8)])

            bias_sb = bpool.tile([P, DOUT], F32)

            xt_sb = {}

            def x_prefetch(t):
                xtile = xpool.tile([P, KT, P], BF16, tag="x", name=f"x{t}")
                nc.gpsimd.dma_start(xtile[:], xt.ap()[t])
                xt_sb[t] = xtile

            # Cold first pair (t0, t1): quarter-chunks interleaved so
            # matmul (k=0, t=0) needs only 256KB of x.
            for t in (0, 1):
                xt_sb[t] = xpool.tile([P, KT, P], BF16, tag="x", name=f"x{t}")
            for c in range(4):
                for t in (0, 1):
                    nc.gpsimd.dma_start(
                        xt_sb[t][:, ts(c, 8), :], xt.ap()[t, :, ts(c, 8), :]
                    )
            x_prefetch(2)
            x_prefetch(3)

            # per-phase W: 4 chunk-tiles of 8 k-tiles each, plain 2D so the
            # moving-operand AP is a simple contiguous slice (a 3D slice
            # into one big blob tile ran every matmul ~20% slower)
            KC = 8                       # k-tiles per W chunk-tile
            wblk = [None] * len(PHASE_PLAN)

            def w_emit(p):
                o0, olen = PHASE_PLAN[p]
                tiles = []
                for c in range(KT // KC):
                    blk = wpool.tile(
                        [P, KC * olen], BF16, tag="w", name=f"w{p}c{c}",
                        padded_shape=[P, KC * OBLK],
                    )
                    # phase 0's first chunk lands as 3 sub-DMAs so matmul
                    # (k=0) needs only 2 k-tiles of W
                    subs = (
                        [(0, 2), (2, 4), (4, 8)] if p == 0 and c == 0
                        else [(0, KC)]
                    )
                    for k0, k1 in subs:
                        eng = nc.sync if (c + k0) % 2 == 0 else nc.scalar
                        eng.dma_start(
                            blk[:, k0 * olen:k1 * olen],
                            wph[p].ap()[
                                :, (c * KC + k0) * olen:(c * KC + k1) * olen
                            ],
                        )
                    tiles.append(blk)
                wblk[p] = tiles

            pairs = [
                (p, tg) for p in range(len(PHASE_PLAN))
                for tg in range(0, NT, 2)
            ]
            for gi, (p, tg) in enumerate(pairs):
                o0, olen = PHASE_PLAN[p]
                nb = olen // BANK
                tpair = (tg, tg + 1)
                if tg == 0:
                    # W blobs: phase 0+1 up front, then p+1 at each phase
                    # start (its pool-slot WAR makes it stream during p).
                    if p == 0:
                        w_emit(0)
                        w_emit(1)
                    elif p + 1 < len(PHASE_PLAN):
                        w_emit(p + 1)
                    nc.gpsimd.dma_start(
                        bias_sb[:, o0:o0 + olen], bias.ap()[:, o0:o0 + olen]
                    )

                # prefetch next pair's x tiles (re-streamed every phase)
                if gi + 1 < len(pairs):
                    for t in (pairs[gi + 1][1], pairs[gi + 1][1] + 1):
                        if t not in xt_sb:
                            x_prefetch(t)

                accs = {
                    t: [
                        psum_pool.tile([P, BANK], F32, tag="acc", name="acc")
                        for _ in range(nb)
                    ]
                    for t in tpair
                }
                for k in range(KT):
                    wc = wblk[p][k // KC]
                    off = (k % KC) * olen
                    for t in tpair:
                        stat = xt_sb[t][:, k, :]
                        for b in range(nb):
                            nc.tensor.matmul(
                                accs[t][b][:],
                                stat,                            # stationary
                                wc[:, off + b * BANK:
                                    off + (b + 1) * BANK],       # moving
                                start=(k == 0),
                                stop=(k == KT - 1),
                            )
                for t in tpair:
                    osb = opool.tile(
                        [P, olen], F32, tag="o", name="o",
                        padded_shape=[P, OBLK],
                    )
                    for b in range(nb):
                        nc.vector.tensor_add(
                            osb[:, ts(b, BANK)],
                            accs[t][b][:],
                            bias_sb[:, o0 + b * BANK:o0 + (b + 1) * BANK],
                        )
                    # outs alternate rings; they sit ahead of later phases'
                    # W triggers in ring order so W WAR blocking never
                    # delays an output write
                    oeng = nc.sync if t % 2 == 0 else nc.scalar
                    oeng.dma_start(
                        out.ap()[ts(t, P), o0:o0 + olen], osb[:]
                    )
                for t in tpair:
                    del xt_sb[t]

    nc.compile()
    return nc


def kernel(x, weight, bias):
    global _NC, LAST_RESULT
    if _NC is None:
        _NC = _build_nc()

    X = np.ascontiguousarray(x.reshape(B * S, DIN))
    wt = weight.T.astype(ml_dtypes.bfloat16)          # [k, o] bf16
    # per-phase contiguous W blobs: [p_, k, o] = wt[k*128+p_, o0+o]
    wblobs = {}
    for p, (o0, olen) in enumerate(PHASE_PLAN):
        blk = wt[:, o0:o0 + olen].reshape(KT, P, olen).transpose(1, 0, 2)
        wblobs[f"w{p}"] = np.ascontiguousarray(blk).reshape(P, KT * olen)
    bias_rep = np.ascontiguousarray(
        np.broadcast_to(bias.astype(np.float32), (P, DOUT))
    )
    in_maps = []
    for c in range(NCORES):
        xc = X[c * T : (c + 1) * T].astype(ml_dtypes.bfloat16)
        # [t-tile, p(=k%128), ks, i(=token%128)]
        xt_c = np.ascontiguousarray(
            xc.reshape(NT, P, KT, P).transpose(0, 3, 2, 1)
        )
        m = {"xt": xt_c, "bias_rep": bias_rep}
        m.update(wblobs)
        in_maps.append(m)

    last_err = None
    for _attempt in range(2):
        try:
            res = run_bass_kernel_spmd(_NC, in_maps, list(range(NCORES)))
            break
        except Exception as e:  # transient NRT device errors: retry once
            last_err = e
    else:
        raise last_err
    LAST_RESULT = res

    out = np.concatenate([res.results[c]["out"] for c in range(NCORES)], axis=0)
    return out.reshape(B, S, DOUT).astype(np.float32, copy=False)


# revision 15
# speedup vs baseline: 1.1904x; 1.1900x over previous
"""Trainium2 kernel for CondensedLinearFineGrainedSparseOp:
    out[b,s,o] = sum_k x[b,s,k] * weight[o,k] + bias[o]
with x [8, 2048, 4096] f32, weight [4096, 4096] f32 (90% zeros, stored
dense), bias [4096] f32 -> out [8, 2048, 4096] f32.

Strategy: data-parallel shard over tokens (B*S = 16384 -> 2048 per core)
across 8 NeuronCores; weight/bias replicated. The unstructured 10%
sparsity is not exploitable on the 128x128 PE array, so each core runs a
dense [2048 x 4096 x 4096] GEMM in bf16 with fp32 PSUM accumulation.
PE roofline: 16t*32k*4096o cols @ 0.4167ns = 874us/core.

Key layout decision: W is pre-tiled on host into one contiguous
partition-major blob per o-phase ([P, KT, olen], 32KB/partition lines).
DMA descriptor-processing overhead, not HBM bandwidth, limits how fast a
single queue delivers data; 1KB-line strided W reads from a [k, o] matrix
deliver ~10x slower than contiguous reads, which starved the PE for the
first ~30us of earlier schedules. With contiguous blobs the cold W chunk
(k0-1, 256KB) lands ~1us after queue start.

Schedule: each o-phase (512, 512, 1024, 1024, 1024 wide) processes t-tile
pairs; within a pair the k-loop interleaves (t, t+1) x (PSUM banks), so a
W k-tile feeds 868ns+ of matmul per touch and per-x-tile first-touch
stalls expose once per pair instead of once per tile. PSUM: 4 banks per
pair double-buffered. W blob for phase p+1 streams during phase p (pool
WAR + in-order rings time it); x tiles re-stream per phase on the SWDGE
queue; outputs alternate the two HWDGE rings behind the current phase's
W chunks but ahead of later phases'.
"""

import numpy as np
import ml_dtypes

import concourse.mybir as mybir
import concourse.tile as tile
from concourse import bacc
from concourse.bass import ts
from concourse.bass_utils import run_bass_kernel_spmd

P = 128
NCORES = 8
B, S, DIN, DOUT = 8, 2048, 4096, 4096
T = B * S // NCORES          # tokens per core
KT = DIN // P                # 32 contraction tiles
NT = T // P                  # 16 token tiles per core
BANK = 512                   # PSUM bank width (f32)
OBLK = 1024                  # W pool slot width

# (o0, olen) per phase; all phases interleave pairs of t-tiles
PHASE_PLAN = [
    (0, 512),
    (512, 512),
    (1024, 1024),
    (2048, 1024),
    (3072, 1024),
]

BF16 = mybir.dt.bfloat16
F32 = mybir.dt.float32

_NC = None
LAST_RESULT = None


def _build_nc():
    nc = bacc.Bacc("TRN2", target_bir_lowering=False, debug=False)
    # x pre-tiled on host: xt[t, p, ks, i] = x[t*128+i, ks*128+p]
    xt = nc.dram_tensor("xt", [NT, P, KT, P], BF16, kind="ExternalInput")
    # W pre-tiled per phase into a contiguous partition-major blob:
    # w{p}[p_, k*olen + o] = weight[o0+o, k*128+p_]  (2D, 32KB lines)
    wph = [
        nc.dram_tensor(f"w{p}", [P, KT * olen], BF16, kind="ExternalInput")
        for p, (o0, olen) in enumerate(PHASE_PLAN)
    ]
    bias = nc.dram_tensor("bias_rep", [P, DOUT], F32, kind="ExternalInput")
    out = nc.dram_tensor("out", [T, DOUT], F32, kind="ExternalOutput")

    with tile.TileContext(nc) as tc:
        with (
            tc.tile_pool(name="wpool", bufs=32) as wpool,
            tc.tile_pool(name="xpool", bufs=5) as xpool,
            tc.tile_pool(name="bpool", bufs=1) as bpool,
            tc.tile_pool(name="opool", bufs=3) as opool,
            tc.tile_pool(name="psum", bufs=8, space="PSUM") as psum_pool,
        ):
            # Tiny warmup DMA on each queue: absorbs cold DGE/queue init.
            for i, eng in enumerate((nc.sync, nc.scalar, nc.gpsimd)):
                wu = bpool.tile([P, 8], F32, tag=f"wu{i}", name=f"wu{i}")
                eng.dma_start(wu[:], bias.ap()[:, ts(i, 8)])

            bias_sb = bpool.tile([P, DOUT], F32)

            xt_sb = {}

            def x_prefetch(t):
                xtile = xpool.tile([P, KT, P], BF16, tag="x", name=f"x{t}")
                nc.gpsimd.dma_start(xtile[:], xt.ap()[t])
                xt_sb[t] = xtile

            # Cold first pair (t0, t1): quarter-chunks interleaved so
            # matmul (k=0, t=0) needs only 256KB of x.
            for t in (0, 1):
                xt_sb[t] = xpool.tile([P, KT, P], BF16, tag="x", name=f"x{t}")
            for c in range(4):
                for t in (0, 1):
                    nc.gpsimd.dma_start(
                        xt_sb[t][:, ts(c, 8), :], xt.ap()[t, :, ts(c, 8), :]
                    )
            x_prefetch(2)
            x_prefetch(3)

            # per-phase W: 4 chunk-tiles of 8 k-tiles each, plain 2D so the
            # moving-operand AP is a simple contiguous slice (a 3D slice
            # into one big blob tile ran every matmul ~20% slower)
            KC = 2                       # k-tiles per W chunk-tile
            wblk = [None] * len(PHASE_PLAN)

            def w_emit(p):
                o0, olen = PHASE_PLAN[p]
                tiles = []
                for c in range(KT // KC):
                    blk = wpool.tile(
                        [P, KC * olen], BF16, tag="w", name=f"w{p}c{c}",
                        padded_shape=[P, KC * OBLK],
                    )
                    # phase 0's first chunk lands as 3 sub-DMAs so matmul
                    # (k=0) needs only 2 k-tiles of W
                    subs = (
                        [(0, 1), (1, 2)] if p == 0 and c == 0
                        else [(0, KC)]
                    )
                    for k0, k1 in subs:
                        eng = nc.sync if (c + k0) % 2 == 0 else nc.scalar
                        eng.dma_start(
                            blk[:, k0 * olen:k1 * olen],
                            wph[p].ap()[
                                :, (c * KC + k0) * olen:(c * KC + k1) * olen
                            ],
                        )
                    tiles.append(blk)
                wblk[p] = tiles

            pairs = [
                (p, tg) for p in range(len(PHASE_PLAN))
                for tg in range(0, NT, 2)
            ]
            for gi, (p, tg) in enumerate(pairs):
                o0, olen = PHASE_PLAN[p]
                nb = olen // BANK
                tpair = (tg, tg + 1)
                if tg == 0:
                    # W blobs: phase 0+1 up front, then p+1 at each phase
                    # start (its pool-slot WAR makes it stream during p).
                    if p == 0:
                        w_emit(0)
                        w_emit(1)
                    elif p + 1 < len(PHASE_PLAN):
                        w_emit(p + 1)
                    nc.gpsimd.dma_start(
                        bias_sb[:, o0:o0 + olen], bias.ap()[:, o0:o0 + olen]
                    )

                # prefetch next pair's x tiles (re-streamed every phase)
                if gi + 1 < len(pairs):
                    for t in (pairs[gi + 1][1], pairs[gi + 1][1] + 1):
                        if t not in xt_sb:
                            x_prefetch(t)

                accs = {
                    t: [
                        psum_pool.tile([P, BANK], F32, tag="acc", name="acc")
                        for _ in range(nb)
                    ]
                    for t in tpair
                }
                for k in range(KT):
                    wc = wblk[p][k // KC]
                    off = (k % KC) * olen
                    for t in tpair:
                        stat = xt_sb[t][:, k, :]
                        for b in range(nb):
                            nc.tensor.matmul(
                                accs[t][b][:],
                                stat,                            # stationary
                                wc[:, off + b * BANK:
                                    off + (b + 1) * BANK],       # moving
                                start=(k == 0),
                                stop=(k == KT - 1),
                            )
                for t in tpair:
                    osb = opool.tile(
                        [P, olen], F32, tag="o", name="o",
                        padded_shape=[P, OBLK],
                    )
                    for b in range(nb):
                        nc.vector.tensor_add(
                            osb[:, ts(b, BANK)],
                            accs[t][b][:],
                            bias_sb[:, o0 + b * BANK:o0 + (b + 1) * BANK],
                        )
                    # outs alternate rings; they sit ahead of later phases'
                    # W triggers in ring order so W WAR blocking never
                    # delays an output write
                    oeng = nc.sync if t % 2 == 0 else nc.scalar
                    oeng.dma_start(
                        out.ap()[ts(t, P), o0:o0 + olen], osb[:]
                    )
                for t in tpair:
                    del xt_sb[t]

    nc.compile()
    return nc


def kernel(x, weight, bias):
    global _NC, LAST_RESULT
    if _NC is None:
        _NC = _build_nc()

    X = np.ascontiguousarray(x.reshape(B * S, DIN))
    wt = weight.T.astype(ml_dtypes.bfloat16)          # [k, o] bf16
    # per-phase contiguous W blobs: [p_, k, o] = wt[k*128+p_, o0+o]
    wblobs = {}
    for p, (o0, olen) in enumerate(PHASE_PLAN):
        blk = wt[:, o0:o0 + olen].reshape(KT, P, olen).transpose(1, 0, 2)
        wblobs[f"w{p}"] = np.ascontiguousarray(blk).reshape(P, KT * olen)
    bias_rep = np.ascontiguousarray(
        np.broadcast_to(bias.astype(np.float32), (P, DOUT))
    )
    in_maps = []
    for c in range(NCORES):
        xc = X[c * T : (c + 1) * T].astype(ml_dtypes.bfloat16)
        # [t-tile, p(=k%128), ks, i(=token%128)]
        xt_c = np.ascontiguousarray(
            xc.reshape(NT, P, KT, P).transpose(0, 3, 2, 1)
        )
        m = {"xt": xt_c, "bias_rep": bias_rep}
        m.update(wblobs)
        in_maps.append(m)

    last_err = None
    for _attempt in range(2):
        try:
            res = run_bass_kernel_spmd(_NC, in_maps, list(range(NCORES)))
            break
        except Exception as e:  # transient NRT device errors: retry once
            last_err = e
    else:
        raise last_err
    LAST_RESULT = res

    out = np.concatenate([res.results[c]["out"] for c in range(NCORES)], axis=0)
    return out.reshape(B, S, DOUT).astype(np.float32, copy=False)


# revision 25
# speedup vs baseline: 1.1942x; 1.0032x over previous
"""Trainium2 kernel for CondensedLinearFineGrainedSparseOp:
    out[b,s,o] = sum_k x[b,s,k] * weight[o,k] + bias[o]
with x [8, 2048, 4096] f32, weight [4096, 4096] f32 (90% zeros, stored
dense), bias [4096] f32 -> out [8, 2048, 4096] f32.

Strategy: data-parallel shard over tokens (B*S = 16384 -> 2048 per core)
across 8 NeuronCores; weight/bias replicated. The unstructured 10%
sparsity is not exploitable on the 128x128 PE array, so each core runs a
dense [2048 x 4096 x 4096] GEMM in bf16 with fp32 PSUM accumulation.
PE roofline: 16t*32k*4096o cols @ 0.4167ns = 874us/core.

Key layout decision: W is pre-tiled on host into one contiguous
partition-major blob per o-phase ([P, KT, olen], 32KB/partition lines).
DMA descriptor-processing overhead, not HBM bandwidth, limits how fast a
single queue delivers data; 1KB-line strided W reads from a [k, o] matrix
deliver ~10x slower than contiguous reads, which starved the PE for the
first ~30us of earlier schedules. With contiguous blobs the cold W chunk
(k0-1, 256KB) lands ~1us after queue start.

Schedule: each o-phase (512, 512, 1024, 1024, 1024 wide) processes t-tile
pairs; within a pair the k-loop interleaves (t, t+1) x (PSUM banks), so a
W k-tile feeds 868ns+ of matmul per touch and per-x-tile first-touch
stalls expose once per pair instead of once per tile. PSUM: 4 banks per
pair double-buffered. W blob for phase p+1 streams during phase p (pool
WAR + in-order rings time it); x tiles re-stream per phase on the SWDGE
queue; outputs alternate the two HWDGE rings behind the current phase's
W chunks but ahead of later phases'.
"""

import numpy as np
import ml_dtypes

import concourse.mybir as mybir
import concourse.tile as tile
from concourse import bacc
from concourse.bass import ts
from concourse.bass_utils import run_bass_kernel_spmd

P = 128
NCORES = 8
B, S, DIN, DOUT = 8, 2048, 4096, 4096
T = B * S // NCORES          # tokens per core
KT = DIN // P                # 32 contraction tiles
NT = T // P                  # 16 token tiles per core
BANK = 512                   # PSUM bank width (f32)
OBLK = 1024                  # W pool slot width

# (o0, olen, ilv): ilv = t-tiles interleaved per k-sweep. The 512-wide
# cold phases interleave 4 t's so a W k-tile feeds 1.7us of matmul
# (~147GB/s demand, chaseable); 1024-wide phases interleave pairs.
PHASE_PLAN = [
    (0, 512, 4),
    (512, 512, 4),
    (1024, 1024, 2),
    (2048, 1024, 2),
    (3072, 1024, 2),
]

BF16 = mybir.dt.bfloat16
F32 = mybir.dt.float32

_NC = None
LAST_RESULT = None


def _build_nc():
    nc = bacc.Bacc("TRN2", target_bir_lowering=False, debug=False)
    # x pre-tiled on host: xt[t, p, ks, i] = x[t*128+i, ks*128+p]
    xt = nc.dram_tensor("xt", [NT, P, KT, P], BF16, kind="ExternalInput")
    # W pre-tiled per phase into a contiguous partition-major blob:
    # w{p}[p_, k*olen + o] = weight[o0+o, k*128+p_]  (2D, 32KB lines)
    wph = [
        nc.dram_tensor(f"w{p}", [P, KT * olen], BF16, kind="ExternalInput")
        for p, (o0, olen, _ilv) in enumerate(PHASE_PLAN)
    ]
    bias = nc.dram_tensor("bias_rep", [P, DOUT], F32, kind="ExternalInput")
    out = nc.dram_tensor("out", [T, DOUT], F32, kind="ExternalOutput")

    with tile.TileContext(nc) as tc:
        with (
            tc.tile_pool(name="wpool", bufs=28) as wpool,
            tc.tile_pool(name="xpool", bufs=8) as xpool,
            tc.tile_pool(name="bpool", bufs=1) as bpool,
            tc.tile_pool(name="opool", bufs=2) as opool,
            tc.tile_pool(name="psum", bufs=8, space="PSUM") as psum_pool,
        ):
            # Single-descriptor warmup DMA on each queue: absorbs cold
            # DGE/queue init. (A [128, 8] warmup is 128 tiny descriptors
            # and clogs the ring for ~10us before the first real DMA.)
            for i, eng in enumerate((nc.sync, nc.scalar, nc.gpsimd)):
                wu = bpool.tile([P, 8], F32, tag=f"wu{i}", name=f"wu{i}")
                eng.dma_start(wu[0:1, :], bias.ap()[0:1, ts(i, 8)])

            bias_sb = bpool.tile([P, DOUT], F32)

            xt_sb = {}

            def x_prefetch(t):
                xtile = xpool.tile([P, KT, P], BF16, tag="x", name=f"x{t}")
                nc.gpsimd.dma_start(xtile[:], xt.ap()[t])
                xt_sb[t] = xtile

            # Cold first group (t0-3): quarter-chunks interleaved across
            # the group so matmul (k=0, t=0) needs only 256KB of x and
            # chunk-rows complete in k order.
            cold = list(range(PHASE_PLAN[0][2]))
            for t in cold:
                xt_sb[t] = xpool.tile([P, KT, P], BF16, tag="x", name=f"x{t}")
            for c in range(4):
                for t in cold:
                    nc.gpsimd.dma_start(
                        xt_sb[t][:, ts(c, 8), :], xt.ap()[t, :, ts(c, 8), :]
                    )

            # per-phase W: 4 chunk-tiles of 8 k-tiles each, plain 2D so the
            # moving-operand AP is a simple contiguous slice (a 3D slice
            # into one big blob tile ran every matmul ~20% slower)
            KC = 2                       # k-tiles per W chunk-tile
            wblk = [None] * len(PHASE_PLAN)

            def w_emit(p):
                o0, olen, _ilv = PHASE_PLAN[p]
                tiles = []
                for c in range(KT // KC):
                    blk = wpool.tile(
                        [P, KC * olen], BF16, tag="w", name=f"w{p}c{c}",
                        padded_shape=[P, KC * OBLK],
                    )
                    # phase 0's first chunk lands as 3 sub-DMAs so matmul
                    # (k=0) needs only 2 k-tiles of W
                    subs = (
                        [(0, 1), (1, KC)] if p == 0 and c == 0
                        else [(0, KC)]
                    )
                    for k0, k1 in subs:
                        eng = nc.sync if (c + k0) % 2 == 0 else nc.scalar
                        eng.dma_start(
                            blk[:, k0 * olen:k1 * olen],
                            wph[p].ap()[
                                :, (c * KC + k0) * olen:(c * KC + k1) * olen
                            ],
                        )
                    tiles.append(blk)
                wblk[p] = tiles

            groups = [
                (p, tg) for p, (_o0, _ol, ilv) in enumerate(PHASE_PLAN)
                for tg in range(0, NT, ilv)
            ]
            for gi, (p, tg) in enumerate(groups):
                o0, olen, ilv = PHASE_PLAN[p]
                nb = olen // BANK
                tpair = tuple(range(tg, tg + ilv))
                if tg == 0:
                    # W blobs: phase 0+1 up front, then p+1 at each phase
                    # start (its pool-slot WAR makes it stream during p).
                    if p == 0:
                        w_emit(0)
                        w_emit(1)
                    elif p + 1 < len(PHASE_PLAN):
                        w_emit(p + 1)
                    nc.gpsimd.dma_start(
                        bias_sb[:, o0:o0 + olen], bias.ap()[:, o0:o0 + olen]
                    )

                # prefetch next group's x tiles (re-streamed every phase)
                if gi + 1 < len(groups):
                    np_, ntg = groups[gi + 1]
                    for t in range(ntg, ntg + PHASE_PLAN[np_][2]):
                        if t not in xt_sb:
                            x_prefetch(t)

                accs = {
                    t: [
                        psum_pool.tile([P, BANK], F32, tag="acc", name="acc")
                        for _ in range(nb)
                    ]
                    for t in tpair
                }
                for k in range(KT):
                    wc = wblk[p][k // KC]
                    off = (k % KC) * olen
                    for t in tpair:
                        stat = xt_sb[t][:, k, :]
                        for b in range(nb):
                            nc.tensor.matmul(
                                accs[t][b][:],
                                stat,                            # stationary
                                wc[:, off + b * BANK:
                                    off + (b + 1) * BANK],       # moving
                                start=(k == 0),
                                stop=(k == KT - 1),
                            )
                for t in tpair:
                    osb = opool.tile(
                        [P, olen], F32, tag="o", name="o",
                        padded_shape=[P, OBLK],
                    )
                    for b in range(nb):
                        nc.vector.tensor_add(
                            osb[:, ts(b, BANK)],
                            accs[t][b][:],
                            bias_sb[:, o0 + b * BANK:o0 + (b + 1) * BANK],
                        )
                    # outs alternate rings; they sit ahead of later phases'
                    # W triggers in ring order so W WAR blocking never
                    # delays an output write
                    oeng = nc.sync if t % 2 == 0 else nc.scalar
                    oeng.dma_start(
                        out.ap()[ts(t, P), o0:o0 + olen], osb[:]
                    )
                for t in tpair:
                    del xt_sb[t]

    nc.compile()
    return nc


def kernel(x, weight, bias):
    global _NC, LAST_RESULT
    if _NC is None:
        _NC = _build_nc()

    X = np.ascontiguousarray(x.reshape(B * S, DIN))
    wt = weight.T.astype(ml_dtypes.bfloat16)          # [k, o] bf16
    # per-phase contiguous W blobs: [p_, k, o] = wt[k*128+p_, o0+o]
    wblobs = {}
    for p, (o0, olen, _ilv) in enumerate(PHASE_PLAN):
        blk = wt[:, o0:o0 + olen].reshape(KT, P, olen).transpose(1, 0, 2)
        wblobs[f"w{p}"] = np.ascontiguousarray(blk).reshape(P, KT * olen)
    bias_rep = np.ascontiguousarray(
        np.broadcast_to(bias.astype(np.float32), (P, DOUT))
    )
    in_maps = []
    for c in range(NCORES):
        xc = X[c * T : (c + 1) * T].astype(ml_dtypes.bfloat16)
        # [t-tile, p(=k%128), ks, i(=token%128)]
        xt_c = np.ascontiguousarray(
            xc.reshape(NT, P, KT, P).transpose(0, 3, 2, 1)
        )
        m = {"xt": xt_c, "bias_rep": bias_rep}
        m.update(wblobs)
        in_maps.append(m)

    last_err = None
    for _attempt in range(2):
        try:
            res = run_bass_kernel_spmd(_NC, in_maps, list(range(NCORES)))
            break
        except Exception as e:  # transient NRT device errors: retry once
            last_err = e
    else:
        raise last_err
    LAST_RESULT = res

    out = np.concatenate([res.results[c]["out"] for c in range(NCORES)], axis=0)
    return out.reshape(B, S, DOUT).astype(np.float32, copy=False)


# revision 28
# speedup vs baseline: 1.2057x; 1.0097x over previous
"""Trainium2 kernel for CondensedLinearFineGrainedSparseOp:
    out[b,s,o] = sum_k x[b,s,k] * weight[o,k] + bias[o]
with x [8, 2048, 4096] f32, weight [4096, 4096] f32 (90% zeros, stored
dense), bias [4096] f32 -> out [8, 2048, 4096] f32.

Strategy: data-parallel shard over tokens (B*S = 16384 -> 2048 per core)
across 8 NeuronCores; weight/bias replicated. The unstructured 10%
sparsity is not exploitable on the 128x128 PE array, so each core runs a
dense [2048 x 4096 x 4096] GEMM in bf16 with fp32 PSUM accumulation.
PE roofline: 16t*32k*4096o cols @ 0.4167ns = 874us/core.

Key layout decision: W is pre-tiled on host into one contiguous
partition-major blob per o-phase ([P, KT, olen], 32KB/partition lines).
DMA descriptor-processing overhead, not HBM bandwidth, limits how fast a
single queue delivers data; 1KB-line strided W reads from a [k, o] matrix
deliver ~10x slower than contiguous reads, which starved the PE for the
first ~30us of earlier schedules. With contiguous blobs the cold W chunk
(k0-1, 256KB) lands ~1us after queue start.

Schedule: each o-phase (512, 512, 1024, 1024, 1024 wide) processes t-tile
pairs; within a pair the k-loop interleaves (t, t+1) x (PSUM banks), so a
W k-tile feeds 868ns+ of matmul per touch and per-x-tile first-touch
stalls expose once per pair instead of once per tile. PSUM: 4 banks per
pair double-buffered. W blob for phase p+1 streams during phase p (pool
WAR + in-order rings time it); x tiles re-stream per phase on the SWDGE
queue; outputs alternate the two HWDGE rings behind the current phase's
W chunks but ahead of later phases'.
"""

import numpy as np
import ml_dtypes

import concourse.mybir as mybir
import concourse.tile as tile
from concourse import bacc
from concourse.bass import ts
from concourse.bass_utils import run_bass_kernel_spmd

P = 128
NCORES = 8
B, S, DIN, DOUT = 8, 2048, 4096, 4096
T = B * S // NCORES          # tokens per core
KT = DIN // P                # 32 contraction tiles
NT = T // P                  # 16 token tiles per core
BANK = 512                   # PSUM bank width (f32)
OBLK = 1024                  # W pool slot width

# (o0, olen, ilv): ilv = t-tiles interleaved per k-sweep. The 512-wide
# cold phases interleave 4 t's so a W k-tile feeds 1.7us of matmul
# (~147GB/s demand, chaseable); 1024-wide phases interleave pairs.
PHASE_PLAN = [
    (0, 512, 4),
    (512, 512, 4),
    (1024, 1024, 2),
    (2048, 1024, 2),
    (3072, 1024, 2),
]

BF16 = mybir.dt.bfloat16
F32 = mybir.dt.float32

_NC = None
LAST_RESULT = None


def _build_nc():
    nc = bacc.Bacc("TRN2", target_bir_lowering=False, debug=False)
    # x pre-tiled on host: xt[t, p, ks, i] = x[t*128+i, ks*128+p]
    xt = nc.dram_tensor("xt", [NT, P, KT, P], BF16, kind="ExternalInput")
    # W pre-tiled per phase into a contiguous partition-major blob:
    # w{p}[p_, k*olen + o] = weight[o0+o, k*128+p_]  (2D, 32KB lines)
    wph = [
        nc.dram_tensor(f"w{p}", [P, KT * olen], BF16, kind="ExternalInput")
        for p, (o0, olen, _ilv) in enumerate(PHASE_PLAN)
    ]
    bias = nc.dram_tensor("bias_rep", [P, DOUT], F32, kind="ExternalInput")
    out = nc.dram_tensor("out", [T, DOUT], F32, kind="ExternalOutput")

    with tile.TileContext(nc) as tc:
        with (
            tc.tile_pool(name="wpool", bufs=28) as wpool,
            tc.tile_pool(name="xpool", bufs=8) as xpool,
            tc.tile_pool(name="bpool", bufs=1) as bpool,
            tc.tile_pool(name="opool", bufs=2) as opool,
            tc.tile_pool(name="psum", bufs=8, space="PSUM") as psum_pool,
        ):
            # Single-descriptor warmup DMA on each queue: absorbs cold
            # DGE/queue init. (A [128, 8] warmup is 128 tiny descriptors
            # and clogs the ring for ~10us before the first real DMA.)
            for i, eng in enumerate((nc.sync, nc.scalar, nc.gpsimd)):
                wu = bpool.tile([P, 8], F32, tag=f"wu{i}", name=f"wu{i}")
                eng.dma_start(wu[0:1, :], bias.ap()[0:1, ts(i, 8)])

            bias_sb = bpool.tile([P, DOUT], F32)

            xt_sb = {}

            def x_prefetch(t):
                xtile = xpool.tile([P, KT, P], BF16, tag="x", name=f"x{t}")
                nc.gpsimd.dma_start(xtile[:], xt.ap()[t])
                xt_sb[t] = xtile

            # Cold first group (t0-3): whole-tile DMAs — SWDGE pays ~1us
            # issue overhead per dma_start, so fewer/bigger beats many
            # interleaved chunks.
            for t in range(PHASE_PLAN[0][2]):
                x_prefetch(t)

            # per-phase W: 4 chunk-tiles of 8 k-tiles each, plain 2D so the
            # moving-operand AP is a simple contiguous slice (a 3D slice
            # into one big blob tile ran every matmul ~20% slower)
            KC = 2                       # k-tiles per W chunk-tile
            wblk = [None] * len(PHASE_PLAN)

            def w_emit(p, eng=None):
                o0, olen, _ilv = PHASE_PLAN[p]
                tiles = []
                for c in range(KT // KC):
                    blk = wpool.tile(
                        [P, KC * olen], BF16, tag="w", name=f"w{p}c{c}",
                        padded_shape=[P, KC * OBLK],
                    )
                    e = eng or (nc.sync if c % 2 == 0 else nc.scalar)
                    e.dma_start(blk[:], wph[p].ap()[:, ts(c, KC * olen)])
                    tiles.append(blk)
                wblk[p] = tiles

            groups = [
                (p, tg) for p, (_o0, _ol, ilv) in enumerate(PHASE_PLAN)
                for tg in range(0, NT, ilv)
            ]
            for gi, (p, tg) in enumerate(groups):
                o0, olen, ilv = PHASE_PLAN[p]
                nb = olen // BANK
                tpair = tuple(range(tg, tg + ilv))
                if tg == 0:
                    # W blobs: phase 0 up front on the HWDGE rings; later
                    # phases' blobs at each phase start (pool-slot WAR +
                    # ring order make them stream during the prior phase).
                    if p == 0:
                        w_emit(0)
                    elif p + 1 < len(PHASE_PLAN):
                        w_emit(p + 1)
                    nc.gpsimd.dma_start(
                        bias_sb[:, o0:o0 + olen], bias.ap()[:, o0:o0 + olen]
                    )
                elif p == 0 and tg == PHASE_PLAN[0][2]:
                    # phase 1's W rides the in-order gpsimd ring BEHIND
                    # group 1's x tiles: keeps the cold window's HWDGE
                    # flood at 4MB and still lands ~80us before needed
                    w_emit(1, eng=nc.gpsimd)

                # prefetch next group's x tiles (re-streamed every phase)
                if gi + 1 < len(groups):
                    np_, ntg = groups[gi + 1]
                    for t in range(ntg, ntg + PHASE_PLAN[np_][2]):
                        if t not in xt_sb:
                            x_prefetch(t)

                accs = {
                    t: [
                        psum_pool.tile([P, BANK], F32, tag="acc", name="acc")
                        for _ in range(nb)
                    ]
                    for t in tpair
                }
                for k in range(KT):
                    wc = wblk[p][k // KC]
                    off = (k % KC) * olen
                    for t in tpair:
                        stat = xt_sb[t][:, k, :]
                        for b in range(nb):
                            nc.tensor.matmul(
                                accs[t][b][:],
                                stat,                            # stationary
                                wc[:, off + b * BANK:
                                    off + (b + 1) * BANK],       # moving
                                start=(k == 0),
                                stop=(k == KT - 1),
                            )
                for t in tpair:
                    osb = opool.tile(
                        [P, olen], F32, tag="o", name="o",
                        padded_shape=[P, OBLK],
                    )
                    for b in range(nb):
                        nc.vector.tensor_add(
                            osb[:, ts(b, BANK)],
                            accs[t][b][:],
                            bias_sb[:, o0 + b * BANK:o0 + (b + 1) * BANK],
                        )
                    # outs alternate rings; they sit ahead of later phases'
                    # W triggers in ring order so W WAR blocking never
                    # delays an output write
                    oeng = nc.sync if t % 2 == 0 else nc.scalar
                    oeng.dma_start(
                        out.ap()[ts(t, P), o0:o0 + olen], osb[:]
                    )
                for t in tpair:
                    del xt_sb[t]

    nc.compile()
    return nc


def kernel(x, weight, bias):
    global _NC, LAST_RESULT
    if _NC is None:
        _NC = _build_nc()

    X = np.ascontiguousarray(x.reshape(B * S, DIN))
    wt = weight.T.astype(ml_dtypes.bfloat16)          # [k, o] bf16
    # per-phase contiguous W blobs: [p_, k, o] = wt[k*128+p_, o0+o]
    wblobs = {}
    for p, (o0, olen, _ilv) in enumerate(PHASE_PLAN):
        blk = wt[:, o0:o0 + olen].reshape(KT, P, olen).transpose(1, 0, 2)
        wblobs[f"w{p}"] = np.ascontiguousarray(blk).reshape(P, KT * olen)
    bias_rep = np.ascontiguousarray(
        np.broadcast_to(bias.astype(np.float32), (P, DOUT))
    )
    in_maps = []
    for c in range(NCORES):
        xc = X[c * T : (c + 1) * T].astype(ml_dtypes.bfloat16)
        # [t-tile, p(=k%128), ks, i(=token%128)]
        xt_c = np.ascontiguousarray(
            xc.reshape(NT, P, KT, P).transpose(0, 3, 2, 1)
        )
        m = {"xt": xt_c, "bias_rep": bias_rep}
        m.update(wblobs)
        in_maps.append(m)

    last_err = None
    for _attempt in range(2):
        try:
            res = run_bass_kernel_spmd(_NC, in_maps, list(range(NCORES)))
            break
        except Exception as e:  # transient NRT device errors: retry once
            last_err = e
    else:
        raise last_err
    LAST_RESULT = res

    out = np.concatenate([res.results[c]["out"] for c in range(NCORES)], axis=0)
    return out.reshape(B, S, DOUT).astype(np.float32, copy=False)
